# revision 11
# baseline (speedup 1.0000x reference)
"""MoE layer (noisy top-2 gating, 8 experts, LN+GELU MLP experts) on 8 trn2 cores.

Sharding: data-parallel over the batch. Each core gets 1024 tokens, all
weights replicated. Per core, on device:
  - gating (fp32 matmuls) -> noisy top-2 -> softmax weights -> gate
  - routing compaction: per-expert token lists via cumsum/one-hot matmuls
  - indirect-DMA gather of selected token rows (bf16)
  - per-expert dense MLP (bf16 matmuls, fp32 LN stats, exact GELU on ACT)
  - indirect-DMA scatter of gate-scaled outputs, final k0+k1 add
Host: shard inputs, run SPMD on 8 cores, concatenate outputs.
"""

import os
import sys

for _p in ("/root/.axon_site", "/root/.axon_site/_ro/trn_rl_repo",
           "/root/.axon_site/_ro/pypackages"):
    if os.path.isdir(_p) and _p not in sys.path:
        sys.path.append(_p)

import numpy as np
import ml_dtypes

import concourse.bass as bass
import concourse.mybir as mybir
import concourse.tile as tile
from concourse import bacc
from concourse.masks import make_identity

F32 = mybir.dt.float32
BF16 = mybir.dt.bfloat16
I32 = mybir.dt.int32
U32 = mybir.dt.uint32

P = 128
B, D, H, E = 8192, 768, 1024, 8
NCORES = 8
BSH = B // NCORES          # 1024 tokens per core
NT = BSH // P              # 8 token tiles
DC = D // P                # 6 d-chunks
HC = H // P                # 8 h-chunks
CAP = 384                  # per-(core, expert) token capacity
CT = CAP // P              # 3 capacity tiles
BIG = float(1 << 20)
LN_EPS = 1e-5
AL = mybir.AluOpType
AF = mybir.ActivationFunctionType


def build_program(debug=False, with_bg=False, with_b1=False, with_b2=False,
                  with_ln_affine=False):
    nc = bacc.Bacc("TRN2", target_bir_lowering=False, debug=False,
                   num_devices=NCORES)

    x = nc.dram_tensor("x", [BSH, D], F32, kind="ExternalInput")
    xb = nc.dram_tensor("xb", [BSH, D], BF16, kind="ExternalInput")
    noise = nc.dram_tensor("noise", [BSH, E], F32, kind="ExternalInput")
    Wg = nc.dram_tensor("Wg", [D, E], F32, kind="ExternalInput")
    Wn = nc.dram_tensor("Wn", [D, E], F32, kind="ExternalInput")
    w1 = nc.dram_tensor("w1", [E, D, H], BF16, kind="ExternalInput")
    w2 = nc.dram_tensor("w2", [E, H, D], BF16, kind="ExternalInput")
    if with_bg:
        bg = nc.dram_tensor("bg", [E], F32, kind="ExternalInput")
        bn = nc.dram_tensor("bn", [E], F32, kind="ExternalInput")
    if with_b1:
        b1 = nc.dram_tensor("b1", [E, H], F32, kind="ExternalInput")
    if with_b2:
        b2 = nc.dram_tensor("b2", [E, D], F32, kind="ExternalInput")
    if with_ln_affine:
        ln_g = nc.dram_tensor("ln_g", [E, H], F32, kind="ExternalInput")
        ln_b = nc.dram_tensor("ln_b", [E, H], F32, kind="ExternalInput")

    moe = nc.dram_tensor("moe", [BSH, D], F32, kind="ExternalOutput")
    cl_out = nc.dram_tensor("cl", [BSH, E], F32, kind="ExternalOutput")
    ti_out = nc.dram_tensor("ti", [BSH, 2], I32, kind="ExternalOutput")
    out01 = nc.dram_tensor("out01", [2 * BSH, D], F32)  # internal scratch

    if debug:
        gate_dbg = nc.dram_tensor("gate_dbg", [BSH, E], F32, kind="ExternalOutput")
        pos_dbg = nc.dram_tensor("pos_dbg", [BSH, E], F32, kind="ExternalOutput")
        list_dbg = nc.dram_tensor("list_dbg", [E, CAP, 4], F32, kind="ExternalOutput")

    from contextlib import ExitStack
    with tile.TileContext(nc) as tc, ExitStack() as ctx:
        const = ctx.enter_context(tc.tile_pool(name="const", bufs=1))
        ps_small = ctx.enter_context(tc.tile_pool(name="ps_small", bufs=2, space="PSUM"))
        psb = ctx.enter_context(tc.tile_pool(name="psb", bufs=3, space="PSUM"))
        gio = ctx.enter_context(tc.tile_pool(name="gio", bufs=2))
        gsb = ctx.enter_context(tc.tile_pool(name="gsb", bufs=NT))
        route = ctx.enter_context(tc.tile_pool(name="route", bufs=NT))
        ohp = ctx.enter_context(tc.tile_pool(name="ohp", bufs=3))
        lists = ctx.enter_context(tc.tile_pool(name="lists", bufs=4))
        idxp = ctx.enter_context(tc.tile_pool(name="idxp", bufs=2 * CT + 2))
        wpool = ctx.enter_context(tc.tile_pool(name="wpool", bufs=2))
        xpool = ctx.enter_context(tc.tile_pool(name="xpool", bufs=3))
        epool = ctx.enter_context(tc.tile_pool(name="epool", bufs=2))
        fpool = ctx.enter_context(tc.tile_pool(name="fpool", bufs=2))
        xtp = ctx.enter_context(tc.tile_pool(name="xtp", bufs=1))

        # ---------------- constants ----------------
        id_f = const.tile([P, P], F32)
        make_identity(nc, id_f[:])

        # LT[t, j] = 1 if t <= j (inclusive lower-tri as lhsT), bf16
        it_tj = const.tile([P, P], I32)
        nc.gpsimd.iota(it_tj[:], pattern=[[1, P]], base=0, channel_multiplier=-1)
        lt_f = const.tile([P, P], F32)
        nc.vector.tensor_scalar(lt_f[:], it_tj[:], 0, None, op0=AL.is_ge)
        lt_b = const.tile([P, P], BF16)
        nc.vector.tensor_copy(lt_b[:], lt_f[:])
        ones_row_f = const.tile([1, P], F32)
        nc.vector.memset(ones_row_f[:], 1.0)
        allones_b = const.tile([P, P], BF16)
        nc.vector.memset(allones_b[:], 1.0)

        itok_i = const.tile([P, 1], I32)
        nc.gpsimd.iota(itok_i[:], pattern=[[0, 1]], base=0, channel_multiplier=1)
        itok_f = const.tile([P, 1], F32)
        nc.vector.tensor_copy(itok_f[:], itok_i[:])

        ie_i = const.tile([P, E], I32)
        nc.gpsimd.iota(ie_i[:], pattern=[[1, E]], base=0, channel_multiplier=0)
        ie_f = const.tile([P, E], F32)
        nc.vector.tensor_copy(ie_f[:], ie_i[:])

        icap_i = const.tile([P, CAP], I32)
        nc.gpsimd.iota(icap_i[:], pattern=[[1, CAP]], base=0, channel_multiplier=0)
        icap_f = const.tile([P, CAP], F32)
        nc.vector.tensor_copy(icap_f[:], icap_i[:])

        # ---------------- load x, build xT (fp32, for gating) ----------------
        xt = xtp.tile([P, DC, BSH], F32)  # xT: [d-part, chunk, tok]
        for t in range(NT):
            xtile = gio.tile([P, D], F32, tag="xin")
            nc.sync.dma_start(out=xtile[:], in_=x[t * P:(t + 1) * P, :])
            for c in range(DC):
                tp = ps_small.tile([P, P], F32, tag="ps_small")
                nc.tensor.transpose(tp[:], xtile[:, c * P:(c + 1) * P], id_f[:])
                nc.scalar.copy(xt[:, c, t * P:(t + 1) * P], tp[:])

        # ---------------- gating matmuls (fp32) ----------------
        wg_sb = const.tile([P, DC, E], F32)
        nc.sync.dma_start(out=wg_sb[:], in_=Wg.rearrange("(c p) e -> p c e", p=P))
        wn_sb = const.tile([P, DC, E], F32)
        nc.sync.dma_start(out=wn_sb[:], in_=Wn.rearrange("(c p) e -> p c e", p=P))

        clT = const.tile([E, BSH], F32, tag="clT")
        nlT = const.tile([E, BSH], F32, tag="nlT")
        for w_sb, lT in ((wg_sb, clT), (wn_sb, nlT)):
            for h2 in range(2):
                lg_ps = psb.tile([E, 512], F32, tag="psb")
                for c in range(DC):
                    nc.tensor.matmul(lg_ps[:], lhsT=w_sb[:, c, :],
                                     rhs=xt[:, c, h2 * 512:(h2 + 1) * 512],
                                     start=(c == 0), stop=(c == DC - 1))
                nc.vector.tensor_copy(lT[:, h2 * 512:(h2 + 1) * 512], lg_ps[:])

        if with_bg:
            bgn_sb = const.tile([1, 2 * E], F32)
            nc.sync.dma_start(out=bgn_sb[:, :E], in_=bg[None, :])
            nc.sync.dma_start(out=bgn_sb[:, E:], in_=bn[None, :])
            bgn_ps = ps_small.tile([P, 2 * E], F32, tag="ps_small")
            nc.tensor.matmul(bgn_ps[:], lhsT=ones_row_f[:], rhs=bgn_sb[:],
                             start=True, stop=True)
            bgn_b = const.tile([P, 2 * E], F32)
            nc.vector.tensor_copy(bgn_b[:], bgn_ps[:])

        # ------- gating, staged so ACT functions batch per LUT table -------
        cl_ts, nl_ts, n_ts, sp_ts = [], [], [], []
        for t in range(NT):
            cl_ps = ps_small.tile([P, E], F32, tag="ps_small")
            nc.tensor.transpose(cl_ps[:], clT[:, t * P:(t + 1) * P], id_f[:8, :8])
            cl_t = gsb.tile([P, E], F32, tag="cl_t")
            if with_bg:
                nc.vector.tensor_add(cl_t[:], cl_ps[:], bgn_b[:, :E])
            else:
                nc.vector.tensor_copy(cl_t[:], cl_ps[:])
            nc.sync.dma_start(out=cl_out[t * P:(t + 1) * P, :], in_=cl_t[:])

            nl_ps = ps_small.tile([P, E], F32, tag="ps_small")
            nc.tensor.transpose(nl_ps[:], nlT[:, t * P:(t + 1) * P], id_f[:8, :8])
            nl_t = gsb.tile([P, E], F32, tag="nl_t")
            if with_bg:
                nc.vector.tensor_add(nl_t[:], nl_ps[:], bgn_b[:, E:])
            else:
                nc.vector.tensor_copy(nl_t[:], nl_ps[:])

            n_t = gsb.tile([P, E], F32, tag="n_t")
            nc.sync.dma_start(out=n_t[:], in_=noise[t * P:(t + 1) * P, :])
            cl_ts.append(cl_t)
            nl_ts.append(nl_t)
            n_ts.append(n_t)

        # softplus(x) = ln(1+exp(x)); all Exp together, then all Ln together
        for t in range(NT):
            sp_t = gsb.tile([P, E], F32, tag="sp_t")
            nc.scalar.activation(sp_t[:], nl_ts[t][:], AF.Exp)
            sp_ts.append(sp_t)
        for t in range(NT):
            nc.vector.tensor_scalar(sp_ts[t][:], sp_ts[t][:], 1.0, None, op0=AL.add)
        for t in range(NT):
            nc.scalar.activation(sp_ts[t][:], sp_ts[t][:], AF.Ln)

        noisys, wks = [], []
        for t in range(NT):
            noisy = gsb.tile([P, E], F32, tag="noisy")
            nc.vector.tensor_mul(noisy[:], n_ts[t][:], sp_ts[t][:])
            nc.vector.tensor_add(noisy[:], noisy[:], cl_ts[t][:])
            noisys.append(noisy)
            wk = gsb.tile([P, 4], F32, tag="wk")  # v-diff, e1, s, unused
            wks.append(wk)

        v8s, i8s = [], []
        for t in range(NT):
            v8 = gsb.tile([P, E], F32, tag="v8")
            nc.vector.max(out=v8[:], in_=noisys[t][:])
            i8 = gsb.tile([P, E], U32, tag="i8")
            nc.vector.max_index(i8[:], v8[:], noisys[t][:])
            ti_t = gsb.tile([P, 2], I32, tag="ti_t")
            nc.vector.tensor_copy(ti_t[:], i8[:, 0:2])
            nc.sync.dma_start(out=ti_out[t * P:(t + 1) * P, :], in_=ti_t[:])
            nc.vector.tensor_tensor(wks[t][:, 0:1], v8[:, 1:2], v8[:, 0:1],
                                    op=AL.subtract)
            v8s.append(v8)
            i8s.append(i8)
        for t in range(NT):
            nc.scalar.activation(wks[t][:, 1:2], wks[t][:, 0:1], AF.Exp)

        mbs, rbigs, posxs = [], [], []
        for t in range(NT):
            wk, i8 = wks[t], i8s[t]
            nc.vector.tensor_scalar(wk[:, 2:3], wk[:, 1:2], 1.0, None, op0=AL.add)
            w0 = gsb.tile([P, 1], F32, tag="w0")
            nc.vector.reciprocal(w0[:], wk[:, 2:3])
            w1c = gsb.tile([P, 1], F32, tag="w1c")
            nc.vector.tensor_mul(w1c[:], wk[:, 1:2], w0[:])

            idx0f = gsb.tile([P, 2], F32, tag="idx0f")
            nc.vector.tensor_copy(idx0f[:], i8[:, 0:2])

            m0 = gsb.tile([P, E], F32, tag="m0")
            nc.vector.tensor_tensor(m0[:], ie_f[:], idx0f[:, 0:1].to_broadcast([P, E]),
                                    op=AL.is_equal)
            m1 = gsb.tile([P, E], F32, tag="m1")
            nc.vector.tensor_tensor(m1[:], ie_f[:], idx0f[:, 1:2].to_broadcast([P, E]),
                                    op=AL.is_equal)
            mS = route.tile([P, E], F32, tag="mS")
            nc.vector.tensor_add(mS[:], m0[:], m1[:])
            mb = route.tile([P, E], BF16, tag="mb")
            nc.vector.tensor_copy(mb[:], mS[:])

            g0 = gsb.tile([P, E], F32, tag="g0")
            nc.vector.tensor_scalar(g0[:], m0[:], w0[:, :1], None, op0=AL.mult)
            gate_t = gsb.tile([P, E], F32, tag="gate_t")
            nc.vector.tensor_scalar(gate_t[:], m1[:], w1c[:, :1], None, op0=AL.mult)
            nc.vector.tensor_add(gate_t[:], gate_t[:], g0[:])
            if debug:
                nc.sync.dma_start(out=gate_dbg[t * P:(t + 1) * P, :], in_=gate_t[:])

            tokid = gsb.tile([P, 1], F32, tag="tokid")
            nc.vector.tensor_scalar(tokid[:], itok_f[:], float(P * t), None, op0=AL.add)
            rb = route.tile([P, E, 4], F32, tag="rb")
            nc.vector.tensor_copy(rb[:, :, 0], tokid[:].to_broadcast([P, E]))
            nc.vector.tensor_copy(rb[:, :, 1], gate_t[:])
            nc.vector.tensor_copy(rb[:, :, 2], m1[:])
            nc.vector.memset(rb[:, :, 3], 1.0)

            mbs.append(mb)
            rbigs.append(rb)
            posxs.append(mS)  # placeholder; replaced below

        # ---------------- per-tile positions ----------------
        # pos[t] = within-tile inclusive cumsum + sum over earlier tiles'
        # per-expert counts (all-ones matmul), then -mask, +BIG on invalid
        for t in range(NT):
            pos_ps = ps_small.tile([P, E], F32, tag="ps_small")
            nc.tensor.matmul(pos_ps[:], lhsT=lt_b[:], rhs=mbs[t][:],
                             start=True, stop=(t == 0))
            for tau in range(t):
                nc.tensor.matmul(pos_ps[:], lhsT=allones_b[:], rhs=mbs[tau][:],
                                 start=False, stop=(tau == t - 1))
            mS = posxs[t]
            pad = gsb.tile([P, E], F32, tag="pad")
            nc.vector.tensor_scalar(pad[:], mS[:], -(BIG + 1.0), BIG,
                                    op0=AL.mult, op1=AL.add)
            posx = route.tile([P, E], F32, tag="posx")
            nc.vector.tensor_add(posx[:], pos_ps[:], pad[:])
            posxs[t] = posx
            if debug:
                nc.sync.dma_start(out=pos_dbg[t * P:(t + 1) * P, :], in_=posx[:])

        # ---------------- per-expert: lists, gather, MLP, scatter ----------
        for e in range(E):
            # compacted list, built transposed: lpT[col, j] via
            # rhs-cols-stationary x one-hot-moving matmuls
            lpT_ps = ps_small.tile([4, CAP], F32, tag="ps_small")
            for t in range(NT):
                oh = ohp.tile([P, CAP], F32, tag="oh")
                nc.vector.tensor_tensor(
                    oh[:], posxs[t][:, e:e + 1].to_broadcast([P, CAP]),
                    icap_f[:], op=AL.is_equal)
                nc.tensor.matmul(lpT_ps[:], lhsT=rbigs[t][:, e, :], rhs=oh[:],
                                 start=(t == 0), stop=(t == NT - 1))
            lpT_sb = lists.tile([4, CAP], F32, tag="lpT")
            nc.scalar.copy(lpT_sb[:], lpT_ps[:])
            list_sb = lists.tile([P, CT, 4], F32, tag="list")
            for cc in range(CT):
                tp = ps_small.tile([P, 4], F32, tag="ps_small")
                nc.tensor.transpose(tp[:], lpT_sb[:, cc * P:(cc + 1) * P],
                                    id_f[:4, :4])
                nc.scalar.copy(list_sb[:, cc, :], tp[:])
                if debug:
                    nc.sync.dma_start(out=list_dbg[e, cc * P:(cc + 1) * P, :],
                                      in_=list_sb[:, cc, :])

            gidxs, sidxs = [], []
            for cc in range(CT):
                gidx = idxp.tile([P, 1], I32, tag="gidx")
                nc.vector.tensor_copy(gidx[:], list_sb[:, cc, 0:1])
                sidx_f = gsb.tile([P, 1], F32, tag="sidx_f")
                # sidx = tokid + BSH*kflag + BIG*(1-valid)
                nc.vector.tensor_scalar(sidx_f[:], list_sb[:, cc, 2:3], float(BSH),
                                        None, op0=AL.mult)
                nc.vector.tensor_add(sidx_f[:], sidx_f[:], list_sb[:, cc, 0:1])
                padv = gsb.tile([P, 1], F32, tag="padv")
                nc.vector.tensor_scalar(padv[:], list_sb[:, cc, 3:4], -BIG, BIG,
                                        op0=AL.mult, op1=AL.add)
                nc.vector.tensor_add(sidx_f[:], sidx_f[:], padv[:])
                sidx = idxp.tile([P, 1], I32, tag="sidx")
                nc.vector.tensor_copy(sidx[:], sidx_f[:])
                gidxs.append(gidx)
                sidxs.append(sidx)

            w1_sb = wpool.tile([P, DC, H], BF16, tag="w1")
            nc.sync.dma_start(out=w1_sb[:], in_=w1[e].rearrange("(c p) h -> p c h", p=P))
            w2_sb = wpool.tile([P, HC, D], BF16, tag="w2")
            nc.sync.dma_start(out=w2_sb[:], in_=w2[e].rearrange("(c p) d -> p c d", p=P))

            if with_b1:
                b1_sb = gio.tile([1, H], F32, tag="b1r")
                nc.sync.dma_start(out=b1_sb[:], in_=b1[e][None, :])
                b1_ps = psb.tile([P, H], F32, tag="psb")
                for h2 in range(2):
                    nc.tensor.matmul(b1_ps[:, h2 * 512:(h2 + 1) * 512],
                                     lhsT=ones_row_f[:],
                                     rhs=b1_sb[:, h2 * 512:(h2 + 1) * 512],
                                     start=True, stop=True)
                b1_b = epool.tile([P, H], F32, tag="b1b")
                nc.vector.tensor_copy(b1_b[:], b1_ps[:])
            if with_ln_affine:
                lng_sb = gio.tile([1, 2 * H], F32, tag="lngr")
                nc.sync.dma_start(out=lng_sb[:, :H], in_=ln_g[e][None, :])
                nc.sync.dma_start(out=lng_sb[:, H:], in_=ln_b[e][None, :])
                lng_ps = psb.tile([P, H], F32, tag="psb")
                for h2 in range(2):
                    nc.tensor.matmul(lng_ps[:, h2 * 512:(h2 + 1) * 512],
                                     lhsT=ones_row_f[:],
                                     rhs=lng_sb[:, h2 * 512:(h2 + 1) * 512],
                                     start=True, stop=True)
                lng_b = epool.tile([P, H], F32, tag="lngb")
                nc.vector.tensor_copy(lng_b[:], lng_ps[:])
                lnb_ps = psb.tile([P, H], F32, tag="psb")
                for h2 in range(2):
                    nc.tensor.matmul(lnb_ps[:, h2 * 512:(h2 + 1) * 512],
                                     lhsT=ones_row_f[:],
                                     rhs=lng_sb[:, H + h2 * 512:H + (h2 + 1) * 512],
                                     start=True, stop=True)
                lnb_b = epool.tile([P, H], F32, tag="lnbb")
                nc.vector.tensor_copy(lnb_b[:], lnb_ps[:])
            if with_b2:
                b2_sb = gio.tile([1, D], F32, tag="b2r")
                nc.sync.dma_start(out=b2_sb[:], in_=b2[e][None, :])
                b2_ps = psb.tile([P, D], F32, tag="psb")
                for (lo, hi) in ((0, 512), (512, D)):
                    nc.tensor.matmul(b2_ps[:, lo:hi], lhsT=ones_row_f[:],
                                     rhs=b2_sb[:, lo:hi], start=True, stop=True)
                b2_b = epool.tile([P, D], F32, tag="b2b")
                nc.vector.tensor_copy(b2_b[:], b2_ps[:])

            for cc in range(CT):
                xg = xpool.tile([P, D], BF16, tag="xg")
                nc.gpsimd.indirect_dma_start(
                    out=xg[:], out_offset=None, in_=xb[:],
                    in_offset=bass.IndirectOffsetOnAxis(ap=gidxs[cc][:, :1], axis=0))
                xgT = xpool.tile([P, DC, P], BF16, tag="xgT")
                nc.scalar.dma_start_transpose(xgT[:], xg[:])

                h_ps = psb.tile([P, H], F32, tag="psb")
                for c in range(DC):
                    for h2 in range(2):
                        nc.tensor.matmul(h_ps[:, h2 * 512:(h2 + 1) * 512],
                                         lhsT=xgT[:, c, :],
                                         rhs=w1_sb[:, c, h2 * 512:(h2 + 1) * 512],
                                         start=(c == 0), stop=(c == DC - 1))

                if with_b1:
                    h_sb = epool.tile([P, H], F32, tag="h_sb")
                    nc.vector.tensor_add(h_sb[:], h_ps[:], b1_b[:])
                    h_src = h_sb
                else:
                    h_src = h_ps

                # LN stats: sum and sum-of-squares via ACT accumulate
                sums = epool.tile([P, 4], F32, tag="sums")
                trash = epool.tile([P, H], BF16, tag="trash")
                nc.scalar.activation(trash[:], h_src[:], AF.Identity,
                                     accum_out=sums[:, 0:1])
                trash2 = epool.tile([P, H], BF16, tag="trash2")
                nc.scalar.activation(trash2[:], h_src[:], AF.Square,
                                     accum_out=sums[:, 1:2])
                # var+eps = Q/H + (S^2)*(-1/H^2) + eps
                nc.vector.tensor_scalar(sums[:, 2:3], sums[:, 0:1],
                                        sums[:, 0:1], None, op0=AL.mult)
                nc.vector.tensor_scalar(sums[:, 2:3], sums[:, 2:3],
                                        -1.0 / (H * H), LN_EPS,
                                        op0=AL.mult, op1=AL.add)
                nc.vector.tensor_scalar(sums[:, 3:4], sums[:, 1:2], 1.0 / H,
                                        None, op0=AL.mult)
                nc.vector.tensor_add(sums[:, 3:4], sums[:, 3:4], sums[:, 2:3])
                # rstd = 1/sqrt(var+eps): Quake seed + 3 Newton steps (DVE only)
                rstd = epool.tile([P, 1], F32, tag="rstd")
                ri = epool.tile([P, 1], I32, tag="ri")
                nc.vector.tensor_scalar(ri[:], sums[:, 3:4].bitcast(I32), 1, None,
                                        op0=AL.arith_shift_right)
                nc.vector.tensor_scalar(ri[:], ri[:], 0x5F3759DF, None,
                                        op0=AL.subtract)
                nc.vector.tensor_scalar(rstd[:].bitcast(I32), ri[:], -1, None,
                                        op0=AL.mult)
                nwt = epool.tile([P, 2], F32, tag="nwt")
                for _ in range(3):
                    nc.vector.tensor_mul(nwt[:, 0:1], rstd[:], rstd[:])
                    nc.vector.tensor_mul(nwt[:, 1:2], nwt[:, 0:1], sums[:, 3:4])
                    nc.vector.tensor_scalar(nwt[:, 1:2], nwt[:, 1:2], -0.5, 1.5,
                                            op0=AL.mult, op1=AL.add)
                    nc.vector.tensor_mul(rstd[:], rstd[:], nwt[:, 1:2])
                nmr = epool.tile([P, 1], F32, tag="nmr")
                nc.vector.tensor_mul(nmr[:], sums[:, 0:1], rstd[:])
                nc.vector.tensor_scalar(nmr[:], nmr[:], -1.0 / H, None, op0=AL.mult)

                gh = epool.tile([P, H], BF16, tag="gh")
                if with_ln_affine:
                    hn = epool.tile([P, H], F32, tag="hn")
                    nc.vector.tensor_scalar(hn[:], h_src[:], rstd[:, :1], nmr[:, :1],
                                            op0=AL.mult, op1=AL.add)
                    nc.vector.tensor_mul(hn[:], hn[:], lng_b[:])
                    nc.vector.tensor_add(hn[:], hn[:], lnb_b[:])
                    nc.scalar.activation(gh[:], hn[:], AF.Gelu)
                else:
                    nc.scalar.activation(gh[:], h_src[:], AF.Gelu,
                                         bias=nmr[:, :1], scale=rstd[:, :1])

                ghT = epool.tile([P, HC, P], BF16, tag="ghT")
                nc.scalar.dma_start_transpose(ghT[:], gh[:])

                o_ps = psb.tile([P, D], F32, tag="psb")
                for hc in range(HC):
                    for (lo, hi) in ((0, 512), (512, D)):
                        nc.tensor.matmul(o_ps[:, lo:hi], lhsT=ghT[:, hc, :],
                                         rhs=w2_sb[:, hc, lo:hi],
                                         start=(hc == 0), stop=(hc == HC - 1))

                ob = fpool.tile([P, D], F32, tag="ob")
                if with_b2:
                    nc.vector.tensor_add(ob[:], o_ps[:], b2_b[:])
                    nc.vector.tensor_scalar(ob[:], ob[:], list_sb[:, cc, 1:2], None,
                                            op0=AL.mult)
                else:
                    nc.scalar.activation(ob[:], o_ps[:], AF.Copy,
                                         scale=list_sb[:, cc, 1:2])
                nc.gpsimd.indirect_dma_start(
                    out=out01[:], out_offset=bass.IndirectOffsetOnAxis(
                        ap=sidxs[cc][:, :1], axis=0),
                    in_=ob[:], in_offset=None,
                    bounds_check=2 * BSH - 1, oob_is_err=False)

        # ---------------- final combine: moe = out01[:BSH] + out01[BSH:] ------
        for t in range(NT):
            a = fpool.tile([P, D], F32, tag="fa")
            nc.sync.dma_start(out=a[:], in_=out01[t * P:(t + 1) * P, :])
            b_ = fpool.tile([P, D], F32, tag="fb")
            nc.sync.dma_start(out=b_[:], in_=out01[BSH + t * P:BSH + (t + 1) * P, :])
            o = fpool.tile([P, D], F32, tag="fo")
            nc.vector.tensor_add(o[:], a[:], b_[:])
            nc.sync.dma_start(out=moe[t * P:(t + 1) * P, :], in_=o[:])

    nc.compile()
    return nc


def _make_in_maps(inputs):
    x = np.asarray(inputs["x"], dtype=np.float32)
    noise = np.asarray(inputs["noise"], dtype=np.float32)
    Wg = np.asarray(inputs["Wg"], dtype=np.float32)
    Wn = np.asarray(inputs["Wn"], dtype=np.float32)
    W1 = np.asarray(inputs["W1"], dtype=np.float32)
    W2 = np.asarray(inputs["W2"], dtype=np.float32)
    xb = x.astype(ml_dtypes.bfloat16)
    w1b = W1.astype(ml_dtypes.bfloat16)
    w2b = W2.astype(ml_dtypes.bfloat16)

    flags = dict(
        with_bg=not (np.all(inputs["bg"] == 0) and np.all(inputs["bn"] == 0)),
        with_b1=not np.all(inputs["b1"] == 0),
        with_b2=not np.all(inputs["b2"] == 0),
        with_ln_affine=not (np.all(inputs["ln_g"] == 1.0)
                            and np.all(inputs["ln_b"] == 0)),
    )

    in_maps = []
    for i in range(NCORES):
        sl = slice(i * BSH, (i + 1) * BSH)
        m = {
            "x": x[sl], "xb": xb[sl], "noise": noise[sl],
            "Wg": Wg, "Wn": Wn, "w1": w1b, "w2": w2b,
        }
        if flags["with_bg"]:
            m["bg"] = np.asarray(inputs["bg"], np.float32)
            m["bn"] = np.asarray(inputs["bn"], np.float32)
        if flags["with_b1"]:
            m["b1"] = np.asarray(inputs["b1"], np.float32)
        if flags["with_b2"]:
            m["b2"] = np.asarray(inputs["b2"], np.float32)
        if flags["with_ln_affine"]:
            m["ln_g"] = np.asarray(inputs["ln_g"], np.float32)
            m["ln_b"] = np.asarray(inputs["ln_b"], np.float32)
        in_maps.append(m)
    return in_maps, flags


_cached = {}


def kernel(**inputs):
    from concourse.bass_utils import run_bass_kernel_spmd

    in_maps, flags = _make_in_maps(inputs)
    key = tuple(sorted(flags.items()))
    if key not in _cached:
        _cached[key] = build_program(debug=False, **flags)
    nc = _cached[key]

    res = run_bass_kernel_spmd(nc, in_maps, list(range(NCORES))).results
    moe = np.concatenate([np.asarray(r["moe"]) for r in res], axis=0)
    cl = np.concatenate([np.asarray(r["cl"]) for r in res], axis=0)
    ti = np.concatenate([np.asarray(r["ti"]) for r in res], axis=0)
    return moe.astype(np.float32), cl.astype(np.float32), ti.astype(np.int32)


if __name__ == "__main__":
    import reference
    inputs = {k: np.asarray(v) for k, v in reference.setup_inputs().items()}
    out = kernel(**inputs)
    print([o.shape for o in out])


# revision 15
# speedup vs baseline: 1.1132x; 1.1132x over previous
"""MoE layer (noisy top-2 gating, 8 experts, LN+GELU MLP experts) on 8 trn2 cores.

Sharding: data-parallel over the batch. Each core gets 1024 tokens, all
weights replicated. Per core, on device:
  - gating (fp32 matmuls) -> noisy top-2 -> softmax weights -> gate
  - routing compaction: per-expert token lists via cumsum/one-hot matmuls
  - indirect-DMA gather of selected token rows (bf16)
  - per-expert dense MLP (bf16 matmuls, fp32 LN stats, exact GELU on ACT)
  - indirect-DMA scatter of gate-scaled outputs, final k0+k1 add
Host: shard inputs, run SPMD on 8 cores, concatenate outputs.
"""

import os
import sys

for _p in ("/root/.axon_site", "/root/.axon_site/_ro/trn_rl_repo",
           "/root/.axon_site/_ro/pypackages"):
    if os.path.isdir(_p) and _p not in sys.path:
        sys.path.append(_p)

import numpy as np
import ml_dtypes

import concourse.bass as bass
import concourse.mybir as mybir
import concourse.tile as tile
from concourse import bacc
from concourse.masks import make_identity

F32 = mybir.dt.float32
BF16 = mybir.dt.bfloat16
I32 = mybir.dt.int32
U32 = mybir.dt.uint32

P = 128
B, D, H, E = 8192, 768, 1024, 8
NCORES = 8
BSH = B // NCORES          # 1024 tokens per core
NT = BSH // P              # 8 token tiles
DC = D // P                # 6 d-chunks
HC = H // P                # 8 h-chunks
CAP = 384                  # per-(core, expert) token capacity
CT = CAP // P              # 3 capacity tiles
BIG = float(1 << 20)
LN_EPS = 1e-5
AL = mybir.AluOpType
AF = mybir.ActivationFunctionType


def build_program(debug=False, with_bg=False, with_b1=False, with_b2=False,
                  with_ln_affine=False):
    nc = bacc.Bacc("TRN2", target_bir_lowering=False, debug=False,
                   num_devices=NCORES)

    x = nc.dram_tensor("x", [BSH, D], F32, kind="ExternalInput")
    xb = nc.dram_tensor("xb", [BSH, D], BF16, kind="ExternalInput")
    noise = nc.dram_tensor("noise", [BSH, E], F32, kind="ExternalInput")
    Wg = nc.dram_tensor("Wg", [D, E], F32, kind="ExternalInput")
    Wn = nc.dram_tensor("Wn", [D, E], F32, kind="ExternalInput")
    w1 = nc.dram_tensor("w1", [E, D, H], BF16, kind="ExternalInput")
    w2 = nc.dram_tensor("w2", [E, H, D], BF16, kind="ExternalInput")
    if with_bg:
        bg = nc.dram_tensor("bg", [E], F32, kind="ExternalInput")
        bn = nc.dram_tensor("bn", [E], F32, kind="ExternalInput")
    if with_b1:
        b1 = nc.dram_tensor("b1", [E, H], F32, kind="ExternalInput")
    if with_b2:
        b2 = nc.dram_tensor("b2", [E, D], F32, kind="ExternalInput")
    if with_ln_affine:
        ln_g = nc.dram_tensor("ln_g", [E, H], F32, kind="ExternalInput")
        ln_b = nc.dram_tensor("ln_b", [E, H], F32, kind="ExternalInput")

    moe = nc.dram_tensor("moe", [BSH, D], F32, kind="ExternalOutput")
    cl_out = nc.dram_tensor("cl", [BSH, E], F32, kind="ExternalOutput")
    ti_out = nc.dram_tensor("ti", [BSH, 2], I32, kind="ExternalOutput")
    out01 = nc.dram_tensor("out01", [2 * BSH, D], F32)  # internal scratch

    if debug:
        gate_dbg = nc.dram_tensor("gate_dbg", [BSH, E], F32, kind="ExternalOutput")
        pos_dbg = nc.dram_tensor("pos_dbg", [BSH, E], F32, kind="ExternalOutput")
        list_dbg = nc.dram_tensor("list_dbg", [E, CAP, 4], F32, kind="ExternalOutput")

    from contextlib import ExitStack
    with tile.TileContext(nc) as tc, ExitStack() as ctx:
        const = ctx.enter_context(tc.tile_pool(name="const", bufs=1))
        ps_small = ctx.enter_context(tc.tile_pool(name="ps_small", bufs=2, space="PSUM"))
        psb = ctx.enter_context(tc.tile_pool(name="psb", bufs=3, space="PSUM"))
        gio = ctx.enter_context(tc.tile_pool(name="gio", bufs=2))
        gsb = ctx.enter_context(tc.tile_pool(name="gsb", bufs=NT))
        route = ctx.enter_context(tc.tile_pool(name="route", bufs=NT))
        ohp = ctx.enter_context(tc.tile_pool(name="ohp", bufs=3))
        lists = ctx.enter_context(tc.tile_pool(name="lists", bufs=4))
        idxp = ctx.enter_context(tc.tile_pool(name="idxp", bufs=2 * CT + 2))
        wpool = ctx.enter_context(tc.tile_pool(name="wpool", bufs=3))
        xpool = ctx.enter_context(tc.tile_pool(name="xpool", bufs=3))
        epool = ctx.enter_context(tc.tile_pool(name="epool", bufs=2))
        fpool = ctx.enter_context(tc.tile_pool(name="fpool", bufs=2))
        xtp = ctx.enter_context(tc.tile_pool(name="xtp", bufs=1))

        # ---------------- constants ----------------
        id_f = const.tile([P, P], F32)
        make_identity(nc, id_f[:])

        # LT[t, j] = 1 if t <= j (inclusive lower-tri as lhsT), bf16
        it_tj = const.tile([P, P], I32)
        nc.gpsimd.iota(it_tj[:], pattern=[[1, P]], base=0, channel_multiplier=-1)
        lt_f = const.tile([P, P], F32)
        nc.vector.tensor_scalar(lt_f[:], it_tj[:], 0, None, op0=AL.is_ge)
        lt_b = const.tile([P, P], BF16)
        nc.vector.tensor_copy(lt_b[:], lt_f[:])
        ones_row_f = const.tile([1, P], F32)
        nc.vector.memset(ones_row_f[:], 1.0)
        allones_b = const.tile([P, P], BF16)
        nc.vector.memset(allones_b[:], 1.0)

        itok_i = const.tile([P, 1], I32)
        nc.gpsimd.iota(itok_i[:], pattern=[[0, 1]], base=0, channel_multiplier=1)
        itok_f = const.tile([P, 1], F32)
        nc.vector.tensor_copy(itok_f[:], itok_i[:])

        ie_i = const.tile([P, E], I32)
        nc.gpsimd.iota(ie_i[:], pattern=[[1, E]], base=0, channel_multiplier=0)
        ie_f = const.tile([P, E], F32)
        nc.vector.tensor_copy(ie_f[:], ie_i[:])

        icap_i = const.tile([P, CAP], I32)
        nc.gpsimd.iota(icap_i[:], pattern=[[1, CAP]], base=0, channel_multiplier=0)
        icap_f = const.tile([P, CAP], F32)
        nc.vector.tensor_copy(icap_f[:], icap_i[:])

        # ---------------- load x, build xT (fp32, for gating) ----------------
        xt = xtp.tile([P, DC, BSH], F32)  # xT: [d-part, chunk, tok]
        for t in range(NT):
            xtile = gio.tile([P, D], F32, tag="xin")
            nc.sync.dma_start(out=xtile[:], in_=x[t * P:(t + 1) * P, :])
            for c in range(DC):
                tp = ps_small.tile([P, P], F32, tag="ps_small")
                nc.tensor.transpose(tp[:], xtile[:, c * P:(c + 1) * P], id_f[:])
                nc.scalar.copy(xt[:, c, t * P:(t + 1) * P], tp[:])

        # ---------------- gating matmuls (fp32) ----------------
        wg_sb = const.tile([P, DC, E], F32)
        nc.sync.dma_start(out=wg_sb[:], in_=Wg.rearrange("(c p) e -> p c e", p=P))
        wn_sb = const.tile([P, DC, E], F32)
        nc.sync.dma_start(out=wn_sb[:], in_=Wn.rearrange("(c p) e -> p c e", p=P))

        clT = const.tile([E, BSH], F32, tag="clT")
        nlT = const.tile([E, BSH], F32, tag="nlT")
        for w_sb, lT in ((wg_sb, clT), (wn_sb, nlT)):
            for h2 in range(2):
                lg_ps = psb.tile([E, 512], F32, tag="psb")
                for c in range(DC):
                    nc.tensor.matmul(lg_ps[:], lhsT=w_sb[:, c, :],
                                     rhs=xt[:, c, h2 * 512:(h2 + 1) * 512],
                                     start=(c == 0), stop=(c == DC - 1))
                nc.vector.tensor_copy(lT[:, h2 * 512:(h2 + 1) * 512], lg_ps[:])

        if with_bg:
            bgn_sb = const.tile([1, 2 * E], F32)
            nc.sync.dma_start(out=bgn_sb[:, :E], in_=bg[None, :])
            nc.sync.dma_start(out=bgn_sb[:, E:], in_=bn[None, :])
            bgn_ps = ps_small.tile([P, 2 * E], F32, tag="ps_small")
            nc.tensor.matmul(bgn_ps[:], lhsT=ones_row_f[:], rhs=bgn_sb[:],
                             start=True, stop=True)
            bgn_b = const.tile([P, 2 * E], F32)
            nc.vector.tensor_copy(bgn_b[:], bgn_ps[:])

        # ------- gating, staged so ACT functions batch per LUT table -------
        cl_ts, nl_ts, n_ts, sp_ts = [], [], [], []
        for t in range(NT):
            cl_ps = ps_small.tile([P, E], F32, tag="ps_small")
            nc.tensor.transpose(cl_ps[:], clT[:, t * P:(t + 1) * P], id_f[:8, :8])
            cl_t = gsb.tile([P, E], F32, tag="cl_t")
            if with_bg:
                nc.vector.tensor_add(cl_t[:], cl_ps[:], bgn_b[:, :E])
            else:
                nc.vector.tensor_copy(cl_t[:], cl_ps[:])
            nc.sync.dma_start(out=cl_out[t * P:(t + 1) * P, :], in_=cl_t[:])

            nl_ps = ps_small.tile([P, E], F32, tag="ps_small")
            nc.tensor.transpose(nl_ps[:], nlT[:, t * P:(t + 1) * P], id_f[:8, :8])
            nl_t = gsb.tile([P, E], F32, tag="nl_t")
            if with_bg:
                nc.vector.tensor_add(nl_t[:], nl_ps[:], bgn_b[:, E:])
            else:
                nc.vector.tensor_copy(nl_t[:], nl_ps[:])

            n_t = gsb.tile([P, E], F32, tag="n_t")
            nc.sync.dma_start(out=n_t[:], in_=noise[t * P:(t + 1) * P, :])
            cl_ts.append(cl_t)
            nl_ts.append(nl_t)
            n_ts.append(n_t)

        # softplus(x) = ln(1+exp(x)); all Exp together, then all Ln together
        for t in range(NT):
            sp_t = gsb.tile([P, E], F32, tag="sp_t")
            nc.scalar.activation(sp_t[:], nl_ts[t][:], AF.Exp)
            sp_ts.append(sp_t)
        for t in range(NT):
            nc.vector.tensor_scalar(sp_ts[t][:], sp_ts[t][:], 1.0, None, op0=AL.add)
        for t in range(NT):
            nc.scalar.activation(sp_ts[t][:], sp_ts[t][:], AF.Ln)

        noisys, wks = [], []
        for t in range(NT):
            noisy = gsb.tile([P, E], F32, tag="noisy")
            nc.vector.tensor_mul(noisy[:], n_ts[t][:], sp_ts[t][:])
            nc.vector.tensor_add(noisy[:], noisy[:], cl_ts[t][:])
            noisys.append(noisy)
            wk = gsb.tile([P, 4], F32, tag="wk")  # v-diff, e1, s, unused
            wks.append(wk)

        v8s, i8s = [], []
        for t in range(NT):
            v8 = gsb.tile([P, E], F32, tag="v8")
            nc.vector.max(out=v8[:], in_=noisys[t][:])
            i8 = gsb.tile([P, E], U32, tag="i8")
            nc.vector.max_index(i8[:], v8[:], noisys[t][:])
            ti_t = gsb.tile([P, 2], I32, tag="ti_t")
            nc.vector.tensor_copy(ti_t[:], i8[:, 0:2])
            nc.sync.dma_start(out=ti_out[t * P:(t + 1) * P, :], in_=ti_t[:])
            nc.vector.tensor_tensor(wks[t][:, 0:1], v8[:, 1:2], v8[:, 0:1],
                                    op=AL.subtract)
            v8s.append(v8)
            i8s.append(i8)
        for t in range(NT):
            nc.scalar.activation(wks[t][:, 1:2], wks[t][:, 0:1], AF.Exp)

        mbs, rbigs, posxs = [], [], []
        for t in range(NT):
            wk, i8 = wks[t], i8s[t]
            nc.vector.tensor_scalar(wk[:, 2:3], wk[:, 1:2], 1.0, None, op0=AL.add)
            w0 = gsb.tile([P, 1], F32, tag="w0")
            nc.vector.reciprocal(w0[:], wk[:, 2:3])
            w1c = gsb.tile([P, 1], F32, tag="w1c")
            nc.vector.tensor_mul(w1c[:], wk[:, 1:2], w0[:])

            idx0f = gsb.tile([P, 2], F32, tag="idx0f")
            nc.vector.tensor_copy(idx0f[:], i8[:, 0:2])

            m0 = gsb.tile([P, E], F32, tag="m0")
            nc.vector.tensor_tensor(m0[:], ie_f[:], idx0f[:, 0:1].to_broadcast([P, E]),
                                    op=AL.is_equal)
            m1 = gsb.tile([P, E], F32, tag="m1")
            nc.vector.tensor_tensor(m1[:], ie_f[:], idx0f[:, 1:2].to_broadcast([P, E]),
                                    op=AL.is_equal)
            mS = route.tile([P, E], F32, tag="mS")
            nc.vector.tensor_add(mS[:], m0[:], m1[:])
            mb = route.tile([P, E], BF16, tag="mb")
            nc.vector.tensor_copy(mb[:], mS[:])

            g0 = gsb.tile([P, E], F32, tag="g0")
            nc.vector.tensor_scalar(g0[:], m0[:], w0[:, :1], None, op0=AL.mult)
            gate_t = gsb.tile([P, E], F32, tag="gate_t")
            nc.vector.tensor_scalar(gate_t[:], m1[:], w1c[:, :1], None, op0=AL.mult)
            nc.vector.tensor_add(gate_t[:], gate_t[:], g0[:])
            if debug:
                nc.sync.dma_start(out=gate_dbg[t * P:(t + 1) * P, :], in_=gate_t[:])

            tokid = gsb.tile([P, 1], F32, tag="tokid")
            nc.vector.tensor_scalar(tokid[:], itok_f[:], float(P * t), None, op0=AL.add)
            rb = route.tile([P, E, 4], F32, tag="rb")
            nc.vector.tensor_copy(rb[:, :, 0], tokid[:].to_broadcast([P, E]))
            nc.vector.tensor_copy(rb[:, :, 1], gate_t[:])
            nc.vector.tensor_copy(rb[:, :, 2], m1[:])
            nc.vector.memset(rb[:, :, 3], 1.0)

            mbs.append(mb)
            rbigs.append(rb)
            posxs.append(mS)  # placeholder; replaced below

        # ---------------- per-tile positions ----------------
        # pos[t] = within-tile inclusive cumsum + sum over earlier tiles'
        # per-expert counts (all-ones matmul), then -mask, +BIG on invalid
        for t in range(NT):
            pos_ps = ps_small.tile([P, E], F32, tag="ps_small")
            nc.tensor.matmul(pos_ps[:], lhsT=lt_b[:], rhs=mbs[t][:],
                             start=True, stop=(t == 0))
            for tau in range(t):
                nc.tensor.matmul(pos_ps[:], lhsT=allones_b[:], rhs=mbs[tau][:],
                                 start=False, stop=(tau == t - 1))
            mS = posxs[t]
            pad = gsb.tile([P, E], F32, tag="pad")
            nc.vector.tensor_scalar(pad[:], mS[:], -(BIG + 1.0), BIG,
                                    op0=AL.mult, op1=AL.add)
            posx = route.tile([P, E], F32, tag="posx")
            nc.vector.tensor_add(posx[:], pos_ps[:], pad[:])
            posxs[t] = posx
            if debug:
                nc.sync.dma_start(out=pos_dbg[t * P:(t + 1) * P, :], in_=posx[:])

        # ---------------- per-expert: lists, gather, MLP, scatter ----------
        for e in range(E):
            # compacted list, built transposed: lpT[col, j] via
            # rhs-cols-stationary x one-hot-moving matmuls
            lpT_ps = ps_small.tile([4, CAP], F32, tag="ps_small")
            for t in range(NT):
                oh = ohp.tile([P, CAP], F32, tag="oh")
                nc.vector.tensor_tensor(
                    oh[:], posxs[t][:, e:e + 1].to_broadcast([P, CAP]),
                    icap_f[:], op=AL.is_equal)
                nc.tensor.matmul(lpT_ps[:], lhsT=rbigs[t][:, e, :], rhs=oh[:],
                                 start=(t == 0), stop=(t == NT - 1))
            lpT_sb = lists.tile([4, CAP], F32, tag="lpT")
            nc.scalar.copy(lpT_sb[:], lpT_ps[:])
            list_sb = lists.tile([P, CT, 4], F32, tag="list")
            for cc in range(CT):
                tp = ps_small.tile([P, 4], F32, tag="ps_small")
                nc.tensor.transpose(tp[:], lpT_sb[:, cc * P:(cc + 1) * P],
                                    id_f[:4, :4])
                nc.scalar.copy(list_sb[:, cc, :], tp[:])
                if debug:
                    nc.sync.dma_start(out=list_dbg[e, cc * P:(cc + 1) * P, :],
                                      in_=list_sb[:, cc, :])

            gidxs, sidxs = [], []
            for cc in range(CT):
                gidx = idxp.tile([P, 1], I32, tag="gidx")
                nc.vector.tensor_copy(gidx[:], list_sb[:, cc, 0:1])
                sidx_f = gsb.tile([P, 1], F32, tag="sidx_f")
                # sidx = tokid + BSH*kflag + BIG*(1-valid)
                nc.vector.tensor_scalar(sidx_f[:], list_sb[:, cc, 2:3], float(BSH),
                                        None, op0=AL.mult)
                nc.vector.tensor_add(sidx_f[:], sidx_f[:], list_sb[:, cc, 0:1])
                padv = gsb.tile([P, 1], F32, tag="padv")
                nc.vector.tensor_scalar(padv[:], list_sb[:, cc, 3:4], -BIG, BIG,
                                        op0=AL.mult, op1=AL.add)
                nc.vector.tensor_add(sidx_f[:], sidx_f[:], padv[:])
                sidx = idxp.tile([P, 1], I32, tag="sidx")
                nc.vector.tensor_copy(sidx[:], sidx_f[:])
                gidxs.append(gidx)
                sidxs.append(sidx)

            w1_sb = wpool.tile([P, DC, H], BF16, tag="w1")
            nc.sync.dma_start(out=w1_sb[:], in_=w1[e].rearrange("(c p) h -> p c h", p=P))
            w2_sb = wpool.tile([P, HC, D], BF16, tag="w2")
            nc.sync.dma_start(out=w2_sb[:], in_=w2[e].rearrange("(c p) d -> p c d", p=P))

            if with_b1:
                b1_sb = gio.tile([1, H], F32, tag="b1r")
                nc.sync.dma_start(out=b1_sb[:], in_=b1[e][None, :])
                b1_ps = psb.tile([P, H], F32, tag="psb")
                for h2 in range(2):
                    nc.tensor.matmul(b1_ps[:, h2 * 512:(h2 + 1) * 512],
                                     lhsT=ones_row_f[:],
                                     rhs=b1_sb[:, h2 * 512:(h2 + 1) * 512],
                                     start=True, stop=True)
                b1_b = epool.tile([P, H], F32, tag="b1b")
                nc.vector.tensor_copy(b1_b[:], b1_ps[:])
            if with_ln_affine:
                lng_sb = gio.tile([1, 2 * H], F32, tag="lngr")
                nc.sync.dma_start(out=lng_sb[:, :H], in_=ln_g[e][None, :])
                nc.sync.dma_start(out=lng_sb[:, H:], in_=ln_b[e][None, :])
                lng_ps = psb.tile([P, H], F32, tag="psb")
                for h2 in range(2):
                    nc.tensor.matmul(lng_ps[:, h2 * 512:(h2 + 1) * 512],
                                     lhsT=ones_row_f[:],
                                     rhs=lng_sb[:, h2 * 512:(h2 + 1) * 512],
                                     start=True, stop=True)
                lng_b = epool.tile([P, H], F32, tag="lngb")
                nc.vector.tensor_copy(lng_b[:], lng_ps[:])
                lnb_ps = psb.tile([P, H], F32, tag="psb")
                for h2 in range(2):
                    nc.tensor.matmul(lnb_ps[:, h2 * 512:(h2 + 1) * 512],
                                     lhsT=ones_row_f[:],
                                     rhs=lng_sb[:, H + h2 * 512:H + (h2 + 1) * 512],
                                     start=True, stop=True)
                lnb_b = epool.tile([P, H], F32, tag="lnbb")
                nc.vector.tensor_copy(lnb_b[:], lnb_ps[:])
            if with_b2:
                b2_sb = gio.tile([1, D], F32, tag="b2r")
                nc.sync.dma_start(out=b2_sb[:], in_=b2[e][None, :])
                b2_ps = psb.tile([P, D], F32, tag="psb")
                for (lo, hi) in ((0, 512), (512, D)):
                    nc.tensor.matmul(b2_ps[:, lo:hi], lhsT=ones_row_f[:],
                                     rhs=b2_sb[:, lo:hi], start=True, stop=True)
                b2_b = epool.tile([P, D], F32, tag="b2b")
                nc.vector.tensor_copy(b2_b[:], b2_ps[:])

            for cc in range(CT):
                xg = xpool.tile([P, D], BF16, tag="xg")
                nc.gpsimd.indirect_dma_start(
                    out=xg[:], out_offset=None, in_=xb[:],
                    in_offset=bass.IndirectOffsetOnAxis(ap=gidxs[cc][:, :1], axis=0))
                xgT = xpool.tile([P, DC, P], BF16, tag="xgT")
                nc.scalar.dma_start_transpose(xgT[:], xg[:])

                h_ps = psb.tile([P, H], F32, tag="psb")
                for c in range(DC):
                    for h2 in range(2):
                        nc.tensor.matmul(h_ps[:, h2 * 512:(h2 + 1) * 512],
                                         lhsT=xgT[:, c, :],
                                         rhs=w1_sb[:, c, h2 * 512:(h2 + 1) * 512],
                                         start=(c == 0), stop=(c == DC - 1))

                # copy h to SBUF (frees the PSUM slot fast) + row sums in
                # the same ACT pass; sum of squares on DVE from the copy
                sums = epool.tile([P, 4], F32, tag="sums")
                h_sb = epool.tile([P, H], F32, tag="h_sb")
                if with_b1:
                    nc.vector.tensor_add(h_sb[:], h_ps[:], b1_b[:])
                    nc.vector.reduce_sum(sums[:, 0:1], h_sb[:],
                                         axis=mybir.AxisListType.X)
                else:
                    nc.scalar.activation(h_sb[:], h_ps[:], AF.Identity,
                                         accum_out=sums[:, 0:1])
                h_src = h_sb
                trash = epool.tile([P, H], BF16, tag="trash")
                nc.vector.tensor_tensor(trash[:], h_sb[:], h_sb[:], op=AL.mult)
                nc.vector.reduce_sum(sums[:, 1:2], trash[:],
                                     axis=mybir.AxisListType.X)
                # var+eps = Q/H + (S^2)*(-1/H^2) + eps
                nc.vector.tensor_scalar(sums[:, 2:3], sums[:, 0:1],
                                        sums[:, 0:1], None, op0=AL.mult)
                nc.vector.tensor_scalar(sums[:, 2:3], sums[:, 2:3],
                                        -1.0 / (H * H), LN_EPS,
                                        op0=AL.mult, op1=AL.add)
                nc.vector.tensor_scalar(sums[:, 3:4], sums[:, 1:2], 1.0 / H,
                                        None, op0=AL.mult)
                nc.vector.tensor_add(sums[:, 3:4], sums[:, 3:4], sums[:, 2:3])
                # rstd = 1/sqrt(var+eps): Quake seed + 3 Newton steps (DVE only)
                rstd = epool.tile([P, 1], F32, tag="rstd")
                ri = epool.tile([P, 1], I32, tag="ri")
                nc.vector.tensor_scalar(ri[:], sums[:, 3:4].bitcast(I32), 1, None,
                                        op0=AL.arith_shift_right)
                nc.vector.tensor_scalar(ri[:], ri[:], 0x5F3759DF, None,
                                        op0=AL.subtract)
                nc.vector.tensor_scalar(rstd[:].bitcast(I32), ri[:], -1, None,
                                        op0=AL.mult)
                nwt = epool.tile([P, 2], F32, tag="nwt")
                for _ in range(2):
                    nc.vector.tensor_mul(nwt[:, 0:1], rstd[:], rstd[:])
                    nc.vector.tensor_mul(nwt[:, 1:2], nwt[:, 0:1], sums[:, 3:4])
                    nc.vector.tensor_scalar(nwt[:, 1:2], nwt[:, 1:2], -0.5, 1.5,
                                            op0=AL.mult, op1=AL.add)
                    nc.vector.tensor_mul(rstd[:], rstd[:], nwt[:, 1:2])
                nmr = epool.tile([P, 1], F32, tag="nmr")
                nc.vector.tensor_mul(nmr[:], sums[:, 0:1], rstd[:])
                nc.vector.tensor_scalar(nmr[:], nmr[:], -1.0 / H, None, op0=AL.mult)

                gh = epool.tile([P, H], BF16, tag="gh")
                if with_ln_affine:
                    hn = epool.tile([P, H], F32, tag="hn")
                    nc.vector.tensor_scalar(hn[:], h_src[:], rstd[:, :1], nmr[:, :1],
                                            op0=AL.mult, op1=AL.add)
                    nc.vector.tensor_mul(hn[:], hn[:], lng_b[:])
                    nc.vector.tensor_add(hn[:], hn[:], lnb_b[:])
                    nc.scalar.activation(gh[:], hn[:], AF.Gelu)
                else:
                    nc.scalar.activation(gh[:], h_src[:], AF.Gelu,
                                         bias=nmr[:, :1], scale=rstd[:, :1])

                ghT = epool.tile([P, HC, P], BF16, tag="ghT")
                nc.scalar.dma_start_transpose(ghT[:], gh[:])

                o_ps = psb.tile([P, D], F32, tag="psb")
                for hc in range(HC):
                    for (lo, hi) in ((0, 512), (512, D)):
                        nc.tensor.matmul(o_ps[:, lo:hi], lhsT=ghT[:, hc, :],
                                         rhs=w2_sb[:, hc, lo:hi],
                                         start=(hc == 0), stop=(hc == HC - 1))

                ob = fpool.tile([P, D], F32, tag="ob")
                if with_b2:
                    nc.vector.tensor_add(ob[:], o_ps[:], b2_b[:])
                    nc.vector.tensor_scalar(ob[:], ob[:], list_sb[:, cc, 1:2], None,
                                            op0=AL.mult)
                else:
                    nc.scalar.activation(ob[:], o_ps[:], AF.Copy,
                                         scale=list_sb[:, cc, 1:2])
                nc.gpsimd.indirect_dma_start(
                    out=out01[:], out_offset=bass.IndirectOffsetOnAxis(
                        ap=sidxs[cc][:, :1], axis=0),
                    in_=ob[:], in_offset=None,
                    bounds_check=2 * BSH - 1, oob_is_err=False)

        # ---------------- final combine: moe = out01[:BSH] + out01[BSH:] ------
        for t in range(NT):
            a = fpool.tile([P, D], F32, tag="fa")
            nc.sync.dma_start(out=a[:], in_=out01[t * P:(t + 1) * P, :])
            b_ = fpool.tile([P, D], F32, tag="fb")
            nc.sync.dma_start(out=b_[:], in_=out01[BSH + t * P:BSH + (t + 1) * P, :])
            o = fpool.tile([P, D], F32, tag="fo")
            nc.vector.tensor_add(o[:], a[:], b_[:])
            nc.sync.dma_start(out=moe[t * P:(t + 1) * P, :], in_=o[:])

    nc.compile()
    return nc


def _make_in_maps(inputs):
    x = np.asarray(inputs["x"], dtype=np.float32)
    noise = np.asarray(inputs["noise"], dtype=np.float32)
    Wg = np.asarray(inputs["Wg"], dtype=np.float32)
    Wn = np.asarray(inputs["Wn"], dtype=np.float32)
    W1 = np.asarray(inputs["W1"], dtype=np.float32)
    W2 = np.asarray(inputs["W2"], dtype=np.float32)
    xb = x.astype(ml_dtypes.bfloat16)
    w1b = W1.astype(ml_dtypes.bfloat16)
    w2b = W2.astype(ml_dtypes.bfloat16)

    flags = dict(
        with_bg=not (np.all(inputs["bg"] == 0) and np.all(inputs["bn"] == 0)),
        with_b1=not np.all(inputs["b1"] == 0),
        with_b2=not np.all(inputs["b2"] == 0),
        with_ln_affine=not (np.all(inputs["ln_g"] == 1.0)
                            and np.all(inputs["ln_b"] == 0)),
    )

    in_maps = []
    for i in range(NCORES):
        sl = slice(i * BSH, (i + 1) * BSH)
        m = {
            "x": x[sl], "xb": xb[sl], "noise": noise[sl],
            "Wg": Wg, "Wn": Wn, "w1": w1b, "w2": w2b,
        }
        if flags["with_bg"]:
            m["bg"] = np.asarray(inputs["bg"], np.float32)
            m["bn"] = np.asarray(inputs["bn"], np.float32)
        if flags["with_b1"]:
            m["b1"] = np.asarray(inputs["b1"], np.float32)
        if flags["with_b2"]:
            m["b2"] = np.asarray(inputs["b2"], np.float32)
        if flags["with_ln_affine"]:
            m["ln_g"] = np.asarray(inputs["ln_g"], np.float32)
            m["ln_b"] = np.asarray(inputs["ln_b"], np.float32)
        in_maps.append(m)
    return in_maps, flags


_cached = {}


def kernel(**inputs):
    from concourse.bass_utils import run_bass_kernel_spmd

    in_maps, flags = _make_in_maps(inputs)
    key = tuple(sorted(flags.items()))
    if key not in _cached:
        _cached[key] = build_program(debug=False, **flags)
    nc = _cached[key]

    res = run_bass_kernel_spmd(nc, in_maps, list(range(NCORES))).results
    moe = np.concatenate([np.asarray(r["moe"]) for r in res], axis=0)
    cl = np.concatenate([np.asarray(r["cl"]) for r in res], axis=0)
    ti = np.concatenate([np.asarray(r["ti"]) for r in res], axis=0)
    return moe.astype(np.float32), cl.astype(np.float32), ti.astype(np.int32)


if __name__ == "__main__":
    import reference
    inputs = {k: np.asarray(v) for k, v in reference.setup_inputs().items()}
    out = kernel(**inputs)
    print([o.shape for o in out])


# revision 17
# speedup vs baseline: 1.5635x; 1.4045x over previous
"""MoE layer (noisy top-2 gating, 8 experts, LN+GELU MLP experts) on 8 trn2 cores.

Sharding: data-parallel over the batch. Each core gets 1024 tokens, all
weights replicated. Per core, on device:
  - gating (fp32 matmuls) -> noisy top-2 -> softmax weights -> gate
  - routing compaction: per-expert token lists via cumsum/one-hot matmuls
  - indirect-DMA gather of selected token rows (bf16)
  - per-expert dense MLP (bf16 matmuls, fp32 LN stats, exact GELU on ACT),
    software-pipelined so the tensor engine never waits on the LN chain
  - indirect-DMA scatter of gate-scaled outputs, final k0+k1 add
Host: shard inputs, run SPMD on 8 cores, concatenate outputs.
"""

import os
import sys

for _p in ("/root/.axon_site", "/root/.axon_site/_ro/trn_rl_repo",
           "/root/.axon_site/_ro/pypackages"):
    if os.path.isdir(_p) and _p not in sys.path:
        sys.path.append(_p)

import numpy as np
import ml_dtypes

import concourse.bass as bass
import concourse.mybir as mybir
import concourse.tile as tile
from concourse import bacc
from concourse.masks import make_identity

F32 = mybir.dt.float32
BF16 = mybir.dt.bfloat16
I32 = mybir.dt.int32
U32 = mybir.dt.uint32

P = 128
B, D, H, E = 8192, 768, 1024, 8
NCORES = 8
BSH = B // NCORES          # 1024 tokens per core
NT = BSH // P              # 8 token tiles
DC = D // P                # 6 d-chunks
HC = H // P                # 8 h-chunks
CAP = 384                  # per-(core, expert) token capacity
CT = CAP // P              # 3 capacity tiles
BIG = float(1 << 20)
LN_EPS = 1e-5
AL = mybir.AluOpType
AF = mybir.ActivationFunctionType


def build_program(debug=False, with_bg=False, with_b1=False, with_b2=False,
                  with_ln_affine=False):
    nc = bacc.Bacc("TRN2", target_bir_lowering=False, debug=False,
                   num_devices=NCORES)

    x = nc.dram_tensor("x", [BSH, D], F32, kind="ExternalInput")
    xb = nc.dram_tensor("xb", [BSH, D], BF16, kind="ExternalInput")
    noise = nc.dram_tensor("noise", [BSH, E], F32, kind="ExternalInput")
    Wg = nc.dram_tensor("Wg", [D, E], F32, kind="ExternalInput")
    Wn = nc.dram_tensor("Wn", [D, E], F32, kind="ExternalInput")
    w1 = nc.dram_tensor("w1", [E, D, H], BF16, kind="ExternalInput")
    w2 = nc.dram_tensor("w2", [E, H, D], BF16, kind="ExternalInput")
    if with_bg:
        bg = nc.dram_tensor("bg", [E], F32, kind="ExternalInput")
        bn = nc.dram_tensor("bn", [E], F32, kind="ExternalInput")
    if with_b1:
        b1 = nc.dram_tensor("b1", [E, H], F32, kind="ExternalInput")
    if with_b2:
        b2 = nc.dram_tensor("b2", [E, D], F32, kind="ExternalInput")
    if with_ln_affine:
        ln_g = nc.dram_tensor("ln_g", [E, H], F32, kind="ExternalInput")
        ln_b = nc.dram_tensor("ln_b", [E, H], F32, kind="ExternalInput")

    moe = nc.dram_tensor("moe", [BSH, D], F32, kind="ExternalOutput")
    cl_out = nc.dram_tensor("cl", [BSH, E], F32, kind="ExternalOutput")
    ti_out = nc.dram_tensor("ti", [BSH, 2], I32, kind="ExternalOutput")
    out01 = nc.dram_tensor("out01", [2 * BSH, D], F32)  # internal scratch

    if debug:
        gate_dbg = nc.dram_tensor("gate_dbg", [BSH, E], F32, kind="ExternalOutput")
        pos_dbg = nc.dram_tensor("pos_dbg", [BSH, E], F32, kind="ExternalOutput")
        list_dbg = nc.dram_tensor("list_dbg", [E, CAP, 4], F32, kind="ExternalOutput")

    from contextlib import ExitStack
    with tile.TileContext(nc) as tc, ExitStack() as ctx:
        const = ctx.enter_context(tc.tile_pool(name="const", bufs=1))
        ps_small = ctx.enter_context(tc.tile_pool(name="ps_small", bufs=2, space="PSUM"))
        psb = ctx.enter_context(tc.tile_pool(name="psb", bufs=3, space="PSUM"))
        lists = ctx.enter_context(tc.tile_pool(name="lists", bufs=E))
        idxp = ctx.enter_context(tc.tile_pool(name="idxp", bufs=2 * E * CT))

        # ---------------- constants ----------------
        id_f = const.tile([P, P], F32)
        make_identity(nc, id_f[:])

        # LT[t, j] = 1 if t <= j (inclusive lower-tri as lhsT), bf16
        it_tj = const.tile([P, P], I32)
        nc.gpsimd.iota(it_tj[:], pattern=[[1, P]], base=0, channel_multiplier=-1)
        lt_f = const.tile([P, P], F32)
        nc.vector.tensor_scalar(lt_f[:], it_tj[:], 0, None, op0=AL.is_ge)
        lt_b = const.tile([P, P], BF16)
        nc.vector.tensor_copy(lt_b[:], lt_f[:])
        ones_row_f = const.tile([1, P], F32)
        nc.vector.memset(ones_row_f[:], 1.0)
        allones_b = const.tile([P, P], BF16)
        nc.vector.memset(allones_b[:], 1.0)

        ie3_i = const.tile([P, NT, E], I32)
        nc.gpsimd.iota(ie3_i[:], pattern=[[0, NT], [1, E]], base=0,
                       channel_multiplier=0)
        ie3_f = const.tile([P, NT, E], F32)
        nc.vector.tensor_copy(ie3_f[:], ie3_i[:])

        tok_i = const.tile([P, NT], I32)
        nc.gpsimd.iota(tok_i[:], pattern=[[P, NT]], base=0, channel_multiplier=1)
        tok_f = const.tile([P, NT], F32)
        nc.vector.tensor_copy(tok_f[:], tok_i[:])

        icap_i = const.tile([P, CAP], I32)
        nc.gpsimd.iota(icap_i[:], pattern=[[1, CAP]], base=0, channel_multiplier=0)
        icap_f = const.tile([P, CAP], F32)
        nc.vector.tensor_copy(icap_f[:], icap_i[:])

        list_sbs, gidxs, sidxs = [], {}, {}
        with ExitStack() as gctx:
            gio = gctx.enter_context(tc.tile_pool(name="gio", bufs=2))
            gsb = gctx.enter_context(tc.tile_pool(name="gsb", bufs=1))
            g8 = gctx.enter_context(tc.tile_pool(name="g8", bufs=NT))
            ohp = gctx.enter_context(tc.tile_pool(name="ohp", bufs=3))
            xtp = gctx.enter_context(tc.tile_pool(name="xtp", bufs=1))

            # ------------ load x, build xT (fp32, for gating) ------------
            xt = xtp.tile([P, DC, BSH], F32)  # xT: [d-part, chunk, tok]
            for t in range(NT):
                xtile = gio.tile([P, D], F32, tag="xin")
                nc.sync.dma_start(out=xtile[:], in_=x[t * P:(t + 1) * P, :])
                for c in range(DC):
                    tp = ps_small.tile([P, P], F32, tag="ps_small")
                    nc.tensor.transpose(tp[:], xtile[:, c * P:(c + 1) * P], id_f[:])
                    nc.scalar.copy(xt[:, c, t * P:(t + 1) * P], tp[:])

            # ---------------- gating matmuls (fp32) ----------------
            wg_sb = gsb.tile([P, DC, E], F32, tag="wg")
            nc.sync.dma_start(out=wg_sb[:], in_=Wg.rearrange("(c p) e -> p c e", p=P))
            wn_sb = gsb.tile([P, DC, E], F32, tag="wn")
            nc.sync.dma_start(out=wn_sb[:], in_=Wn.rearrange("(c p) e -> p c e", p=P))

            clT = gsb.tile([E, BSH], F32, tag="clT")
            nlT = gsb.tile([E, BSH], F32, tag="nlT")
            for w_sb, lT in ((wg_sb, clT), (wn_sb, nlT)):
                for h2 in range(2):
                    lg_ps = psb.tile([E, 512], F32, tag="psb")
                    for c in range(DC):
                        nc.tensor.matmul(lg_ps[:], lhsT=w_sb[:, c, :],
                                         rhs=xt[:, c, h2 * 512:(h2 + 1) * 512],
                                         start=(c == 0), stop=(c == DC - 1))
                    nc.vector.tensor_copy(lT[:, h2 * 512:(h2 + 1) * 512], lg_ps[:])

            if with_bg:
                bgn_sb = gsb.tile([1, 2 * E], F32, tag="bgn")
                nc.sync.dma_start(out=bgn_sb[:, :E], in_=bg[None, :])
                nc.sync.dma_start(out=bgn_sb[:, E:], in_=bn[None, :])
                bgn_ps = ps_small.tile([P, 2 * E], F32, tag="ps_small")
                nc.tensor.matmul(bgn_ps[:], lhsT=ones_row_f[:], rhs=bgn_sb[:],
                                 start=True, stop=True)
                bgn_b = gsb.tile([P, 2 * E], F32, tag="bgnb")
                nc.vector.tensor_copy(bgn_b[:], bgn_ps[:])

            # ------- gating: batched elementwise over [128, NT*E] -------
            cl_all = gsb.tile([P, NT, E], F32, tag="cl_all")
            nl_all = gsb.tile([P, NT, E], F32, tag="nl_all")
            for t in range(NT):
                cl_ps = ps_small.tile([P, E], F32, tag="ps_small")
                nc.tensor.transpose(cl_ps[:], clT[:, t * P:(t + 1) * P], id_f[:8, :8])
                if with_bg:
                    nc.vector.tensor_add(cl_all[:, t, :], cl_ps[:], bgn_b[:, :E])
                else:
                    nc.scalar.copy(cl_all[:, t, :], cl_ps[:])
                nl_ps = ps_small.tile([P, E], F32, tag="ps_small")
                nc.tensor.transpose(nl_ps[:], nlT[:, t * P:(t + 1) * P], id_f[:8, :8])
                if with_bg:
                    nc.vector.tensor_add(nl_all[:, t, :], nl_ps[:], bgn_b[:, E:])
                else:
                    nc.scalar.copy(nl_all[:, t, :], nl_ps[:])
            nc.sync.dma_start(out=cl_out.rearrange("(t p) e -> p t e", p=P),
                              in_=cl_all[:])

            n_all = gsb.tile([P, NT, E], F32, tag="n_all")
            nc.sync.dma_start(out=n_all[:],
                              in_=noise.rearrange("(t p) e -> p t e", p=P))

            # softplus(x) = ln(1+exp(x)) on the whole [128, NT*E] block
            sp_all = gsb.tile([P, NT, E], F32, tag="sp_all")
            nc.scalar.activation(sp_all[:], nl_all[:], AF.Exp)
            nc.vector.tensor_scalar(sp_all[:], sp_all[:], 1.0, None, op0=AL.add)
            nc.scalar.activation(sp_all[:], sp_all[:], AF.Ln)

            noisy_all = gsb.tile([P, NT, E], F32, tag="noisy_all")
            nc.vector.tensor_mul(noisy_all[:], n_all[:], sp_all[:])
            nc.vector.tensor_add(noisy_all[:], noisy_all[:], cl_all[:])

            i8_all = gsb.tile([P, NT, 2], U32, tag="i8_all")
            wk_all = gsb.tile([P, NT, 2], F32, tag="wk_all")
            for t in range(NT):
                v8 = g8.tile([P, E], F32, tag="v8")
                nc.vector.max(out=v8[:], in_=noisy_all[:, t, :])
                i8 = g8.tile([P, E], U32, tag="i8")
                nc.vector.max_index(i8[:], v8[:], noisy_all[:, t, :])
                nc.vector.tensor_copy(i8_all[:, t, :], i8[:, 0:2])
                nc.vector.tensor_tensor(wk_all[:, t, 0:1], v8[:, 1:2], v8[:, 0:1],
                                        op=AL.subtract)
            nc.sync.dma_start(out=ti_out.rearrange("(t p) k -> p t k", p=P),
                              in_=i8_all[:].bitcast(I32))

            # softmax over top-2: w1 = e1/(1+e1), w0 = 1 - w1 = 1/(1+e1)
            nc.scalar.activation(wk_all[:, :, 1], wk_all[:, :, 0], AF.Exp)
            w01 = gsb.tile([P, NT, 2], F32, tag="w01")  # cols: w0, w1
            nc.vector.tensor_scalar(w01[:, :, 1], wk_all[:, :, 1], 1.0, None,
                                    op0=AL.add)
            nc.vector.reciprocal(w01[:, :, 0], w01[:, :, 1])
            nc.vector.tensor_mul(w01[:, :, 1], wk_all[:, :, 1], w01[:, :, 0])

            idxf = gsb.tile([P, NT, 2], F32, tag="idxf")
            nc.vector.tensor_copy(idxf[:], i8_all[:])

            m0_all = gsb.tile([P, NT, E], F32, tag="m0_all")
            nc.vector.tensor_tensor(m0_all[:], ie3_f[:],
                                    idxf[:, :, 0:1].to_broadcast([P, NT, E]),
                                    op=AL.is_equal)
            m1_all = gsb.tile([P, NT, E], F32, tag="m1_all")
            nc.vector.tensor_tensor(m1_all[:], ie3_f[:],
                                    idxf[:, :, 1:2].to_broadcast([P, NT, E]),
                                    op=AL.is_equal)
            mS_all = gsb.tile([P, NT, E], F32, tag="mS_all")
            nc.vector.tensor_add(mS_all[:], m0_all[:], m1_all[:])
            mb_all = gsb.tile([P, NT, E], BF16, tag="mb_all")
            nc.vector.tensor_copy(mb_all[:], mS_all[:])

            gate_all = gsb.tile([P, NT, E], F32, tag="gate_all")
            nc.vector.tensor_mul(gate_all[:], m0_all[:],
                                 w01[:, :, 0:1].to_broadcast([P, NT, E]))
            g1_all = gsb.tile([P, NT, E], F32, tag="g1_all")
            nc.vector.tensor_mul(g1_all[:], m1_all[:],
                                 w01[:, :, 1:2].to_broadcast([P, NT, E]))
            nc.vector.tensor_add(gate_all[:], gate_all[:], g1_all[:])
            if debug:
                nc.sync.dma_start(out=gate_dbg.rearrange("(t p) e -> p t e", p=P),
                                  in_=gate_all[:])

            rb_all = gsb.tile([P, NT, E, 4], F32, tag="rb_all")
            nc.vector.tensor_copy(
                rb_all[:, :, :, 0],
                tok_f[:].rearrange("p (t o) -> p t o", o=1).to_broadcast([P, NT, E]))
            nc.vector.tensor_copy(rb_all[:, :, :, 1], gate_all[:])
            nc.vector.tensor_copy(rb_all[:, :, :, 2], m1_all[:])
            nc.vector.memset(rb_all[:, :, :, 3], 1.0)

            pad_all = gsb.tile([P, NT, E], F32, tag="pad_all")
            nc.vector.tensor_scalar(pad_all[:], mS_all[:], -(BIG + 1.0), BIG,
                                    op0=AL.mult, op1=AL.add)

            # per-tile positions: inclusive cumsum + earlier-tile counts
            posx_all = gsb.tile([P, NT, E], F32, tag="posx_all")
            for t in range(NT):
                pos_ps = ps_small.tile([P, E], F32, tag="ps_small")
                nc.tensor.matmul(pos_ps[:], lhsT=lt_b[:], rhs=mb_all[:, t, :],
                                 start=True, stop=(t == 0))
                for tau in range(t):
                    nc.tensor.matmul(pos_ps[:], lhsT=allones_b[:],
                                     rhs=mb_all[:, tau, :],
                                     start=False, stop=(tau == t - 1))
                nc.vector.tensor_add(posx_all[:, t, :], pos_ps[:], pad_all[:, t, :])
            if debug:
                nc.sync.dma_start(out=pos_dbg.rearrange("(t p) e -> p t e", p=P),
                                  in_=posx_all[:])

            # ---- per-expert compacted lists (transposed build) + indices ----
            for e in range(E):
                lpT_ps = ps_small.tile([4, CAP], F32, tag="ps_small")
                for t in range(NT):
                    oh = ohp.tile([P, CAP], F32, tag="oh")
                    nc.vector.tensor_tensor(
                        oh[:], posx_all[:, t, e:e + 1].to_broadcast([P, CAP]),
                        icap_f[:], op=AL.is_equal)
                    nc.tensor.matmul(lpT_ps[:], lhsT=rb_all[:, t, e, :], rhs=oh[:],
                                     start=(t == 0), stop=(t == NT - 1))
                lpT_sb = lists.tile([4, CAP], F32, tag="lpT")
                nc.scalar.copy(lpT_sb[:], lpT_ps[:])
                list_sb = lists.tile([P, CT, 4], F32, tag="list")
                for cc in range(CT):
                    tp = ps_small.tile([P, 4], F32, tag="ps_small")
                    nc.tensor.transpose(tp[:], lpT_sb[:, cc * P:(cc + 1) * P],
                                        id_f[:4, :4])
                    nc.scalar.copy(list_sb[:, cc, :], tp[:])
                    if debug:
                        nc.sync.dma_start(out=list_dbg[e, cc * P:(cc + 1) * P, :],
                                          in_=list_sb[:, cc, :])
                list_sbs.append(list_sb)

                for cc in range(CT):
                    gidx = idxp.tile([P, 1], I32, tag="gidx")
                    nc.vector.tensor_copy(gidx[:], list_sb[:, cc, 0:1])
                    sidx_f = g8.tile([P, 2], F32, tag="sidx_f")
                    # sidx = tokid + BSH*kflag + BIG*(1-valid)
                    nc.vector.tensor_scalar(sidx_f[:, 0:1], list_sb[:, cc, 2:3],
                                            float(BSH), None, op0=AL.mult)
                    nc.vector.tensor_add(sidx_f[:, 0:1], sidx_f[:, 0:1],
                                         list_sb[:, cc, 0:1])
                    nc.vector.tensor_scalar(sidx_f[:, 1:2], list_sb[:, cc, 3:4],
                                            -BIG, BIG, op0=AL.mult, op1=AL.add)
                    nc.vector.tensor_add(sidx_f[:, 0:1], sidx_f[:, 0:1],
                                         sidx_f[:, 1:2])
                    sidx = idxp.tile([P, 1], I32, tag="sidx")
                    nc.vector.tensor_copy(sidx[:], sidx_f[:, 0:1])
                    gidxs[(e, cc)] = gidx
                    sidxs[(e, cc)] = sidx

        # ---------- expert MLPs, software-pipelined (distance 2) ----------
        wpool = ctx.enter_context(tc.tile_pool(name="wpool", bufs=3))
        xpool = ctx.enter_context(tc.tile_pool(name="xpool", bufs=4))
        epool = ctx.enter_context(tc.tile_pool(name="epool", bufs=3))
        fpool = ctx.enter_context(tc.tile_pool(name="fpool", bufs=2))

        NIT = E * CT
        w_sbs = {}
        bias_sbs = {}

        def load_expert_weights(e):
            w1_sb = wpool.tile([P, DC, H], BF16, tag="w1")
            nc.sync.dma_start(out=w1_sb[:],
                              in_=w1[e].rearrange("(c p) h -> p c h", p=P))
            w2_sb = wpool.tile([P, HC, D], BF16, tag="w2")
            nc.sync.dma_start(out=w2_sb[:],
                              in_=w2[e].rearrange("(c p) d -> p c d", p=P))
            w_sbs[e] = (w1_sb, w2_sb)
            ex = {}
            if with_b1:
                b1_sb = epool.tile([1, H], F32, tag="b1r")
                nc.sync.dma_start(out=b1_sb[:], in_=b1[e][None, :])
                b1_ps = psb.tile([P, H], F32, tag="psb")
                for h2 in range(2):
                    nc.tensor.matmul(b1_ps[:, h2 * 512:(h2 + 1) * 512],
                                     lhsT=ones_row_f[:],
                                     rhs=b1_sb[:, h2 * 512:(h2 + 1) * 512],
                                     start=True, stop=True)
                ex["b1"] = epool.tile([P, H], F32, tag="b1b")
                nc.vector.tensor_copy(ex["b1"][:], b1_ps[:])
            if with_ln_affine:
                lng_sb = epool.tile([1, 2 * H], F32, tag="lngr")
                nc.sync.dma_start(out=lng_sb[:, :H], in_=ln_g[e][None, :])
                nc.sync.dma_start(out=lng_sb[:, H:], in_=ln_b[e][None, :])
                for nm, off in (("lng", 0), ("lnb", H)):
                    t_ps = psb.tile([P, H], F32, tag="psb")
                    for h2 in range(2):
                        nc.tensor.matmul(t_ps[:, h2 * 512:(h2 + 1) * 512],
                                         lhsT=ones_row_f[:],
                                         rhs=lng_sb[:, off + h2 * 512:
                                                    off + (h2 + 1) * 512],
                                         start=True, stop=True)
                    ex[nm] = epool.tile([P, H], F32, tag=nm + "b")
                    nc.vector.tensor_copy(ex[nm][:], t_ps[:])
            if with_b2:
                b2_sb = epool.tile([1, D], F32, tag="b2r")
                nc.sync.dma_start(out=b2_sb[:], in_=b2[e][None, :])
                b2_ps = psb.tile([P, D], F32, tag="psb")
                for (lo, hi) in ((0, 512), (512, D)):
                    nc.tensor.matmul(b2_ps[:, lo:hi], lhsT=ones_row_f[:],
                                     rhs=b2_sb[:, lo:hi], start=True, stop=True)
                ex["b2"] = epool.tile([P, D], F32, tag="b2b")
                nc.vector.tensor_copy(ex["b2"][:], b2_ps[:])
            bias_sbs[e] = ex

        st = {}  # per-iteration live ghT tiles

        def stage_a(it):
            """gather + transpose + mm1 + LN stats + gelu + ghT transpose"""
            e, cc = divmod(it, CT)
            if cc == 0 and e not in w_sbs:
                load_expert_weights(e)
            w1_sb, _ = w_sbs[e]
            ex = bias_sbs[e]

            xg = xpool.tile([P, D], BF16, tag="xg")
            nc.gpsimd.indirect_dma_start(
                out=xg[:], out_offset=None, in_=xb[:],
                in_offset=bass.IndirectOffsetOnAxis(ap=gidxs[(e, cc)][:, :1], axis=0))
            xgT = xpool.tile([P, DC, P], BF16, tag="xgT")
            nc.scalar.dma_start_transpose(xgT[:], xg[:])

            h_ps = psb.tile([P, H], F32, tag="psb")
            for c in range(DC):
                for h2 in range(2):
                    nc.tensor.matmul(h_ps[:, h2 * 512:(h2 + 1) * 512],
                                     lhsT=xgT[:, c, :],
                                     rhs=w1_sb[:, c, h2 * 512:(h2 + 1) * 512],
                                     start=(c == 0), stop=(c == DC - 1))

            # copy h to SBUF (frees the PSUM slot fast) + row sum in one ACT op
            sums = epool.tile([P, 4], F32, tag="sums")
            h_sb = epool.tile([P, H], F32, tag="h_sb")
            if with_b1:
                nc.vector.tensor_add(h_sb[:], h_ps[:], ex["b1"][:])
                nc.vector.reduce_sum(sums[:, 0:1], h_sb[:],
                                     axis=mybir.AxisListType.X)
            else:
                nc.scalar.activation(h_sb[:], h_ps[:], AF.Identity,
                                     accum_out=sums[:, 0:1])
            trash = epool.tile([P, H], F32, tag="trash")
            nc.vector.tensor_tensor(trash[:], h_sb[:], h_sb[:], op=AL.mult)
            nc.vector.reduce_sum(sums[:, 1:2], trash[:], axis=mybir.AxisListType.X)

            # var+eps = Q/H + S^2 * (-1/H^2) + eps
            nc.vector.tensor_scalar(sums[:, 2:3], sums[:, 0:1], sums[:, 0:1],
                                    None, op0=AL.mult)
            nc.vector.tensor_scalar(sums[:, 2:3], sums[:, 2:3], -1.0 / (H * H),
                                    LN_EPS, op0=AL.mult, op1=AL.add)
            nc.vector.tensor_scalar(sums[:, 3:4], sums[:, 1:2], 1.0 / H, None,
                                    op0=AL.mult)
            nc.vector.tensor_add(sums[:, 3:4], sums[:, 3:4], sums[:, 2:3])
            # rstd = 1/sqrt(var+eps): Quake seed + Newton steps (DVE only)
            rstd = epool.tile([P, 1], F32, tag="rstd")
            ri = epool.tile([P, 1], I32, tag="ri")
            nc.vector.tensor_scalar(ri[:], sums[:, 3:4].bitcast(I32), 1, None,
                                    op0=AL.arith_shift_right)
            nc.vector.tensor_scalar(ri[:], ri[:], 0x5F3759DF, None,
                                    op0=AL.subtract)
            nc.vector.tensor_scalar(rstd[:].bitcast(I32), ri[:], -1, None,
                                    op0=AL.mult)
            nwt = epool.tile([P, 2], F32, tag="nwt")
            for _ in range(3):
                nc.vector.tensor_mul(nwt[:, 0:1], rstd[:], rstd[:])
                nc.vector.tensor_mul(nwt[:, 1:2], nwt[:, 0:1], sums[:, 3:4])
                nc.vector.tensor_scalar(nwt[:, 1:2], nwt[:, 1:2], -0.5, 1.5,
                                        op0=AL.mult, op1=AL.add)
                nc.vector.tensor_mul(rstd[:], rstd[:], nwt[:, 1:2])
            nmr = epool.tile([P, 1], F32, tag="nmr")
            nc.vector.tensor_mul(nmr[:], sums[:, 0:1], rstd[:])
            nc.vector.tensor_scalar(nmr[:], nmr[:], -1.0 / H, None, op0=AL.mult)

            gh = epool.tile([P, H], BF16, tag="gh")
            if with_ln_affine:
                hn = epool.tile([P, H], F32, tag="hn")
                nc.vector.tensor_scalar(hn[:], h_sb[:], rstd[:, :1], nmr[:, :1],
                                        op0=AL.mult, op1=AL.add)
                nc.vector.tensor_mul(hn[:], hn[:], ex["lng"][:])
                nc.vector.tensor_add(hn[:], hn[:], ex["lnb"][:])
                nc.scalar.activation(gh[:], hn[:], AF.Gelu)
            else:
                nc.scalar.activation(gh[:], h_sb[:], AF.Gelu,
                                     bias=nmr[:, :1], scale=rstd[:, :1])

            ghT = epool.tile([P, HC, P], BF16, tag="ghT")
            nc.scalar.dma_start_transpose(ghT[:], gh[:])
            st[it] = ghT

        def stage_b(it):
            """mm2 + gate-scale + scatter"""
            e, cc = divmod(it, CT)
            _, w2_sb = w_sbs[e]
            ghT = st.pop(it)
            o_ps = psb.tile([P, D], F32, tag="psb")
            for hc in range(HC):
                for (lo, hi) in ((0, 512), (512, D)):
                    nc.tensor.matmul(o_ps[:, lo:hi], lhsT=ghT[:, hc, :],
                                     rhs=w2_sb[:, hc, lo:hi],
                                     start=(hc == 0), stop=(hc == HC - 1))
            ob = fpool.tile([P, D], F32, tag="ob")
            if with_b2:
                nc.vector.tensor_add(ob[:], o_ps[:], bias_sbs[e]["b2"][:])
                nc.vector.tensor_scalar(ob[:], ob[:], list_sbs[e][:, cc, 1:2],
                                        None, op0=AL.mult)
            else:
                nc.scalar.activation(ob[:], o_ps[:], AF.Copy,
                                     scale=list_sbs[e][:, cc, 1:2])
            nc.gpsimd.indirect_dma_start(
                out=out01[:], out_offset=bass.IndirectOffsetOnAxis(
                    ap=sidxs[(e, cc)][:, :1], axis=0),
                in_=ob[:], in_offset=None,
                bounds_check=2 * BSH - 1, oob_is_err=False)

        DIST = 2
        for step in range(NIT + DIST):
            if step < NIT:
                stage_a(step)
            if step >= DIST:
                stage_b(step - DIST)

        # -------- final combine: moe = out01[:BSH] + out01[BSH:] --------
        for t in range(NT):
            a = fpool.tile([P, D], F32, tag="fa")
            nc.sync.dma_start(out=a[:], in_=out01[t * P:(t + 1) * P, :])
            b_ = fpool.tile([P, D], F32, tag="fb")
            nc.sync.dma_start(out=b_[:], in_=out01[BSH + t * P:BSH + (t + 1) * P, :])
            o = fpool.tile([P, D], F32, tag="fo")
            nc.vector.tensor_add(o[:], a[:], b_[:])
            nc.sync.dma_start(out=moe[t * P:(t + 1) * P, :], in_=o[:])

    nc.compile()
    return nc


def _make_in_maps(inputs):
    x = np.asarray(inputs["x"], dtype=np.float32)
    noise = np.asarray(inputs["noise"], dtype=np.float32)
    Wg = np.asarray(inputs["Wg"], dtype=np.float32)
    Wn = np.asarray(inputs["Wn"], dtype=np.float32)
    W1 = np.asarray(inputs["W1"], dtype=np.float32)
    W2 = np.asarray(inputs["W2"], dtype=np.float32)
    xb = x.astype(ml_dtypes.bfloat16)
    w1b = W1.astype(ml_dtypes.bfloat16)
    w2b = W2.astype(ml_dtypes.bfloat16)

    flags = dict(
        with_bg=not (np.all(inputs["bg"] == 0) and np.all(inputs["bn"] == 0)),
        with_b1=not np.all(inputs["b1"] == 0),
        with_b2=not np.all(inputs["b2"] == 0),
        with_ln_affine=not (np.all(inputs["ln_g"] == 1.0)
                            and np.all(inputs["ln_b"] == 0)),
    )

    in_maps = []
    for i in range(NCORES):
        sl = slice(i * BSH, (i + 1) * BSH)
        m = {
            "x": x[sl], "xb": xb[sl], "noise": noise[sl],
            "Wg": Wg, "Wn": Wn, "w1": w1b, "w2": w2b,
        }
        if flags["with_bg"]:
            m["bg"] = np.asarray(inputs["bg"], np.float32)
            m["bn"] = np.asarray(inputs["bn"], np.float32)
        if flags["with_b1"]:
            m["b1"] = np.asarray(inputs["b1"], np.float32)
        if flags["with_b2"]:
            m["b2"] = np.asarray(inputs["b2"], np.float32)
        if flags["with_ln_affine"]:
            m["ln_g"] = np.asarray(inputs["ln_g"], np.float32)
            m["ln_b"] = np.asarray(inputs["ln_b"], np.float32)
        in_maps.append(m)
    return in_maps, flags


_cached = {}


def kernel(**inputs):
    from concourse.bass_utils import run_bass_kernel_spmd

    in_maps, flags = _make_in_maps(inputs)
    key = tuple(sorted(flags.items()))
    if key not in _cached:
        _cached[key] = build_program(debug=False, **flags)
    nc = _cached[key]

    res = run_bass_kernel_spmd(nc, in_maps, list(range(NCORES))).results
    moe = np.concatenate([np.asarray(r["moe"]) for r in res], axis=0)
    cl = np.concatenate([np.asarray(r["cl"]) for r in res], axis=0)
    ti = np.concatenate([np.asarray(r["ti"]) for r in res], axis=0)
    return moe.astype(np.float32), cl.astype(np.float32), ti.astype(np.int32)


if __name__ == "__main__":
    import reference
    inputs = {k: np.asarray(v) for k, v in reference.setup_inputs().items()}
    out = kernel(**inputs)
    print([o.shape for o in out])
